# revision 3
# baseline (speedup 1.0000x reference)
"""BVH Qwen router adapter kernel for 8x Trainium2 NeuronCores.

Strategy: data-parallel over tokens (2048 tokens/core), router weights
replicated. Host pre-transposes hidden_states to [H, T] so the contraction
dim lands on SBUF partitions; both router matmuls run as one fused fp32
matmul chain per 128-token tile (lhsT = x^T chunk, rhs = [bvh^T | orig^T],
N=256). Softmax / top-32 candidate mask / top-8 select all run per-token
along the free dim on DVE/ACT while PE continues with the next tile.

Top-k semantics match jax.lax.top_k: nc.vector.max / max_index /
match_replace resolve duplicate values by ascending index (first
unmatched occurrence), the same stable tie-break top_k uses. Ranking for
the final top-8 uses the computed full_probs values themselves (as the
reference does), so ordering agrees with the reference up to fp32
rounding of the logits.
"""

import numpy as np

TOKENS = 16384
HIDDEN = 2048
E = 128          # num experts
TOPK = 8
NCAND = 32
NCORES = 8
P = 128          # partitions
TL = TOKENS // NCORES        # tokens per core
CH = HIDDEN // P             # 16 contraction chunks
STW = 512                    # token super-tile width (one DMA per h-chunk)
NEG = -1.0e30

_nc_cache = {}


def _build(tl=TL):
    import concourse.bacc as bacc
    import concourse.mybir as mybir
    from concourse.tile import TileContext

    f32 = mybir.dt.float32
    u32 = mybir.dt.uint32
    i32 = mybir.dt.int32
    X = mybir.AxisListType.X
    Exp = mybir.ActivationFunctionType.Exp

    n_st = tl // STW             # super tiles
    n_sub = STW // P             # 128-token tiles per super tile

    nc = bacc.Bacc()
    x_d = nc.dram_tensor("xt", [HIDDEN, tl], f32, kind="ExternalInput")
    w_d = nc.dram_tensor("wt", [HIDDEN, 2 * E], f32, kind="ExternalInput")
    p_d = nc.dram_tensor("probs", [tl, E], f32, kind="ExternalOutput")
    v_d = nc.dram_tensor("topv", [tl, TOPK], f32, kind="ExternalOutput")
    i_d = nc.dram_tensor("topi", [tl, TOPK], i32, kind="ExternalOutput")

    with TileContext(nc) as tc:
        with (
            tc.tile_pool(name="wpool", bufs=1) as wpool,
            tc.tile_pool(name="xpool", bufs=2 * CH) as xpool,
            tc.tile_pool(name="spool", bufs=4) as spool,
            tc.tile_pool(name="psum", bufs=8, space="PSUM") as psum_pool,
        ):
            wr = w_d.rearrange("(c p) e -> p c e", p=P)
            xr = x_d.rearrange("(c p) t -> p c t", p=P)

            # Startup: interleave weight-chunk and first-token-tile DMAs in
            # fine [128,128] pieces so the first matmuls start after ~0.4MB
            # instead of after the whole 6MB prologue.
            wt = []
            x0 = [[None] * CH for _ in range(n_sub)]
            for c in range(CH):
                wc = wpool.tile([P, 2 * E], f32, tag=f"w{c}", name=f"w{c}")
                nc.sync.dma_start(wc[:], wr[:, c, :])
                wt.append(wc)
                xc = xpool.tile([P, P], f32, tag="x0", name=f"x0_0_{c}", bufs=CH)
                nc.sync.dma_start(xc[:], xr[:, c, 0:P])
                x0[0][c] = xc
            for sub in range(1, n_sub):
                for c in range(CH):
                    xc = xpool.tile([P, P], f32, tag=f"x0s{sub}",
                                    name=f"x0_{sub}_{c}", bufs=CH)
                    nc.sync.dma_start(xc[:], xr[:, c, sub * P:(sub + 1) * P])
                    x0[sub][c] = xc

            for st in range(n_st):
                if st == 0:
                    xt = None
                else:
                    xt = []
                    for c in range(CH):
                        xc = xpool.tile([P, STW], f32, tag="x", name=f"x_{st}_{c}")
                        nc.sync.dma_start(xc[:], xr[:, c, st * STW:(st + 1) * STW])
                        xt.append(xc)
                for sub in range(n_sub):
                    t0 = st * n_sub + sub
                    rows = slice(t0 * P, (t0 + 1) * P)

                    ps = psum_pool.tile([P, 2 * E], f32, tag="ps", name=f"ps{t0}")
                    for c in range(CH):
                        lhsT = x0[sub][c][:] if st == 0 else \
                            xt[c][:, sub * P:(sub + 1) * P]
                        nc.tensor.matmul(
                            ps[:], lhsT, wt[c][:],
                            start=(c == 0), stop=(c == CH - 1),
                        )

                    # softmax over original-router logits (cols E:2E)
                    negm = spool.tile([P, 1], f32, tag="negm", name=f"negm{t0}")
                    nc.vector.reduce_max(negm[:], ps[:, E:2 * E], axis=X, negate=True)
                    pe_t = spool.tile([P, E], f32, tag="pe", name=f"pe{t0}")
                    s_t = spool.tile([P, 1], f32, tag="S", name=f"S{t0}")
                    nc.scalar.activation(pe_t[:], ps[:, E:2 * E], Exp,
                                         bias=negm[:], scale=1.0, accum_out=s_t[:])
                    rs_t = spool.tile([P, 1], f32, tag="rS", name=f"rS{t0}")
                    nc.vector.reciprocal(rs_t[:], s_t[:])
                    probs = spool.tile([P, E], f32, tag="probs", name=f"probs{t0}")
                    nc.scalar.mul(probs[:], pe_t[:], rs_t[:])
                    nc.sync.dma_start(p_d[rows, :], probs[:])

                    # bvh top-32 candidate mask (rank by bvh logits; softmax
                    # is monotone so the candidate set matches the reference)
                    bvh = spool.tile([P, E], f32, tag="bvh", name=f"bvh{t0}")
                    nc.scalar.copy(bvh[:], ps[:, 0:E])
                    mx8 = spool.tile([P, 8], f32, tag="mx8", name=f"mx8{t0}")
                    for _ in range(NCAND // 8):
                        nc.vector.max(out=mx8[:], in_=bvh[:])
                        nc.vector.match_replace(out=bvh[:], in_to_replace=mx8[:],
                                                in_values=bvh[:], imm_value=NEG)
                    mask = spool.tile([P, E], f32, tag="mask", name=f"mask{t0}")
                    nc.vector.tensor_scalar(mask[:], bvh[:], NEG, None,
                                            op0=mybir.AluOpType.is_equal)
                    masked = spool.tile([P, E], f32, tag="masked", name=f"masked{t0}")
                    nc.vector.tensor_mul(masked[:], probs[:], mask[:])

                    # top-8 among candidates by full probs
                    tv8 = spool.tile([P, 8], f32, tag="tv8", name=f"tv8{t0}")
                    nc.vector.max(out=tv8[:], in_=masked[:])
                    idx8 = spool.tile([P, 8], u32, tag="idx8", name=f"idx8{t0}")
                    nc.vector.max_index(idx8[:], tv8[:], masked[:])
                    s8 = spool.tile([P, 1], f32, tag="s8", name=f"s8{t0}")
                    nc.vector.reduce_sum(s8[:], tv8[:], axis=X)
                    rs8 = spool.tile([P, 1], f32, tag="rs8", name=f"rs8{t0}")
                    nc.vector.reciprocal(rs8[:], s8[:])
                    topv = spool.tile([P, TOPK], f32, tag="topv", name=f"topv{t0}")
                    nc.scalar.mul(topv[:], tv8[:], rs8[:])
                    nc.sync.dma_start(v_d[rows, :], topv[:])
                    topi = spool.tile([P, TOPK], i32, tag="topi", name=f"topi{t0}")
                    nc.vector.tensor_copy(topi[:], idx8[:])
                    nc.sync.dma_start(i_d[rows, :], topi[:])

    nc.finalize()
    return nc


def get_nc(tl=TL):
    if tl not in _nc_cache:
        _nc_cache[tl] = _build(tl)
    return _nc_cache[tl]


def kernel(hidden_states, original_weight, bvh_weight, trace=False):
    from concourse.bass_utils import run_bass_kernel_spmd

    nc = get_nc()

    x = np.ascontiguousarray(np.asarray(hidden_states, dtype=np.float32))
    xT = x.T  # [H, T]
    wcat = np.ascontiguousarray(
        np.concatenate(
            [np.asarray(bvh_weight, np.float32).T,
             np.asarray(original_weight, np.float32).T],
            axis=1,
        )
    )  # [H, 2E]

    in_maps = [
        {"xt": np.ascontiguousarray(xT[:, c * TL:(c + 1) * TL]), "wt": wcat}
        for c in range(NCORES)
    ]
    r = run_bass_kernel_spmd(nc, in_maps, core_ids=list(range(NCORES)),
                             trace=trace)
    res = r.results
    full_probs = np.concatenate([m["probs"] for m in res], axis=0)
    top_vals = np.concatenate([m["topv"] for m in res], axis=0)
    top_idx = np.concatenate([m["topi"] for m in res], axis=0)
    if trace:
        kernel.last_result = r
    return full_probs, top_vals, top_idx


# revision 4
# speedup vs baseline: 1.2143x; 1.2143x over previous
"""BVH Qwen router adapter kernel for 8x Trainium2 NeuronCores.

Strategy: data-parallel over tokens (2048 tokens/core), router weights
replicated. Host pre-transposes hidden_states to [H, T] so the contraction
dim lands on SBUF partitions; both router matmuls run as one fused fp32
matmul chain per 128-token tile (lhsT = x^T chunk, rhs = [bvh^T | orig^T],
N=256). Softmax / top-32 candidate mask / top-8 select all run per-token
along the free dim on DVE/ACT while PE continues with the next tile.

Top-k semantics match jax.lax.top_k: nc.vector.max / max_index /
match_replace resolve duplicate values by ascending index (first
unmatched occurrence), the same stable tie-break top_k uses. Ranking for
the final top-8 uses the computed full_probs values themselves (as the
reference does), so ordering agrees with the reference up to fp32
rounding of the logits.

DMA note: each dma_start costs ~650ns of sync-sequencer time regardless
of size, so transfers are batched: x in [128,512] chunk tiles, weights in
4 grouped loads interleaved with the first x chunks, outputs gathered per
512-token super-tile.
"""

import numpy as np

TOKENS = 16384
HIDDEN = 2048
E = 128          # num experts
TOPK = 8
NCAND = 32
NCORES = 8
P = 128          # partitions
TL = TOKENS // NCORES        # tokens per core
CH = HIDDEN // P             # 16 contraction chunks
STW = 512                    # token super-tile width (one DMA per h-chunk)
WG = 4                       # weight chunks per grouped DMA
NEG = -1.0e30

_nc_cache = {}


def _build(tl=TL):
    import concourse.bacc as bacc
    import concourse.mybir as mybir
    from concourse.tile import TileContext

    f32 = mybir.dt.float32
    u32 = mybir.dt.uint32
    i32 = mybir.dt.int32
    X = mybir.AxisListType.X
    Exp = mybir.ActivationFunctionType.Exp

    n_st = tl // STW             # super tiles
    n_sub = STW // P             # 128-token tiles per super tile

    nc = bacc.Bacc()
    x_d = nc.dram_tensor("xt", [HIDDEN, tl], f32, kind="ExternalInput")
    w_d = nc.dram_tensor("wt", [HIDDEN, 2 * E], f32, kind="ExternalInput")
    p_d = nc.dram_tensor("probs", [tl, E], f32, kind="ExternalOutput")
    v_d = nc.dram_tensor("topv", [tl, TOPK], f32, kind="ExternalOutput")
    i_d = nc.dram_tensor("topi", [tl, TOPK], i32, kind="ExternalOutput")

    with TileContext(nc) as tc:
        with (
            tc.tile_pool(name="wpool", bufs=1) as wpool,
            tc.tile_pool(name="xpool", bufs=2 * CH) as xpool,
            tc.tile_pool(name="opool", bufs=2) as opool,
            tc.tile_pool(name="spool", bufs=4) as spool,
            tc.tile_pool(name="psum", bufs=8, space="PSUM") as psum_pool,
        ):
            # w grouped as [P, WG, 2E] tiles; x in [P, STW] chunk tiles.
            wr = w_d.rearrange("(g q p) e -> p g q e", p=P, q=WG)
            xr = x_d.rearrange("(c p) t -> p c t", p=P)

            wt = []          # wt[c] -> AP of weight chunk c
            xt0 = []         # first super-tile chunk tiles
            for g in range(CH // WG):
                wg = wpool.tile([P, WG, 2 * E], f32, tag=f"w{g}", name=f"w{g}")
                nc.sync.dma_start(wg[:], wr[:, g, :, :])
                for q in range(WG):
                    wt.append(wg[:, q, :])
                for q in range(WG):
                    c = g * WG + q
                    xc = xpool.tile([P, STW], f32, tag="x", name=f"x_0_{c}")
                    nc.sync.dma_start(xc[:], xr[:, c, 0:STW])
                    xt0.append(xc)

            for st in range(n_st):
                if st == 0:
                    xt = xt0
                else:
                    xt = []
                    for c in range(CH):
                        xc = xpool.tile([P, STW], f32, tag="x", name=f"x_{st}_{c}")
                        nc.sync.dma_start(xc[:], xr[:, c, st * STW:(st + 1) * STW])
                        xt.append(xc)

                # per-super-tile output staging (one DMA per tensor)
                probs_o = opool.tile([P, n_sub, E], f32, tag="probs_o",
                                     name=f"probs_o{st}")
                topv_o = opool.tile([P, n_sub, TOPK], f32, tag="topv_o",
                                    name=f"topv_o{st}")
                topi_o = opool.tile([P, n_sub, TOPK], i32, tag="topi_o",
                                    name=f"topi_o{st}")

                for sub in range(n_sub):
                    t0 = st * n_sub + sub

                    ps = psum_pool.tile([P, 2 * E], f32, tag="ps", name=f"ps{t0}")
                    for c in range(CH):
                        nc.tensor.matmul(
                            ps[:], xt[c][:, sub * P:(sub + 1) * P], wt[c],
                            start=(c == 0), stop=(c == CH - 1),
                        )

                    # softmax over original-router logits (cols E:2E)
                    negm = spool.tile([P, 1], f32, tag="negm", name=f"negm{t0}")
                    nc.vector.reduce_max(negm[:], ps[:, E:2 * E], axis=X, negate=True)
                    pe_t = spool.tile([P, E], f32, tag="pe", name=f"pe{t0}")
                    s_t = spool.tile([P, 1], f32, tag="S", name=f"S{t0}")
                    nc.scalar.activation(pe_t[:], ps[:, E:2 * E], Exp,
                                         bias=negm[:], scale=1.0, accum_out=s_t[:])
                    rs_t = spool.tile([P, 1], f32, tag="rS", name=f"rS{t0}")
                    nc.vector.reciprocal(rs_t[:], s_t[:])
                    probs = probs_o[:, sub, :]
                    nc.scalar.mul(probs, pe_t[:], rs_t[:])

                    # bvh top-32 candidate mask (rank by bvh logits; softmax
                    # is monotone so the candidate set matches the reference)
                    bvh = spool.tile([P, E], f32, tag="bvh", name=f"bvh{t0}")
                    nc.scalar.copy(bvh[:], ps[:, 0:E])
                    mx8 = spool.tile([P, 8], f32, tag="mx8", name=f"mx8{t0}")
                    for _ in range(NCAND // 8):
                        nc.vector.max(out=mx8[:], in_=bvh[:])
                        nc.vector.match_replace(out=bvh[:], in_to_replace=mx8[:],
                                                in_values=bvh[:], imm_value=NEG)
                    mask = spool.tile([P, E], f32, tag="mask", name=f"mask{t0}")
                    nc.vector.tensor_scalar(mask[:], bvh[:], NEG, None,
                                            op0=mybir.AluOpType.is_equal)
                    masked = spool.tile([P, E], f32, tag="masked", name=f"masked{t0}")
                    nc.vector.tensor_mul(masked[:], probs, mask[:])

                    # top-8 among candidates by full probs
                    tv8 = spool.tile([P, 8], f32, tag="tv8", name=f"tv8{t0}")
                    nc.vector.max(out=tv8[:], in_=masked[:])
                    idx8 = spool.tile([P, 8], u32, tag="idx8", name=f"idx8{t0}")
                    nc.vector.max_index(idx8[:], tv8[:], masked[:])
                    s8 = spool.tile([P, 1], f32, tag="s8", name=f"s8{t0}")
                    nc.vector.reduce_sum(s8[:], tv8[:], axis=X)
                    rs8 = spool.tile([P, 1], f32, tag="rs8", name=f"rs8{t0}")
                    nc.vector.reciprocal(rs8[:], s8[:])
                    nc.scalar.mul(topv_o[:, sub, :], tv8[:], rs8[:])
                    nc.vector.tensor_copy(topi_o[:, sub, :], idx8[:])

                rows = slice(st * STW, (st + 1) * STW)
                nc.sync.dma_start(
                    p_d[rows, :].rearrange("(s p) e -> p s e", p=P), probs_o[:])
                nc.sync.dma_start(
                    v_d[rows, :].rearrange("(s p) k -> p s k", p=P), topv_o[:])
                nc.sync.dma_start(
                    i_d[rows, :].rearrange("(s p) k -> p s k", p=P), topi_o[:])

    nc.finalize()
    return nc


def get_nc(tl=TL):
    if tl not in _nc_cache:
        _nc_cache[tl] = _build(tl)
    return _nc_cache[tl]


def kernel(hidden_states, original_weight, bvh_weight, trace=False):
    from concourse.bass_utils import run_bass_kernel_spmd

    nc = get_nc()

    x = np.ascontiguousarray(np.asarray(hidden_states, dtype=np.float32))
    xT = x.T  # [H, T]
    wcat = np.ascontiguousarray(
        np.concatenate(
            [np.asarray(bvh_weight, np.float32).T,
             np.asarray(original_weight, np.float32).T],
            axis=1,
        )
    )  # [H, 2E]

    in_maps = [
        {"xt": np.ascontiguousarray(xT[:, c * TL:(c + 1) * TL]), "wt": wcat}
        for c in range(NCORES)
    ]
    r = run_bass_kernel_spmd(nc, in_maps, core_ids=list(range(NCORES)),
                             trace=trace)
    res = r.results
    full_probs = np.concatenate([m["probs"] for m in res], axis=0)
    top_vals = np.concatenate([m["topv"] for m in res], axis=0)
    top_idx = np.concatenate([m["topi"] for m in res], axis=0)
    if trace:
        kernel.last_result = r
    return full_probs, top_vals, top_idx
